# revision 11
# baseline (speedup 1.0000x reference)
"""Trainium2 Bass kernel for nn_MultiHeadLinearAttention — fp8 DoubleRow v6.

Data-parallel over batch across 8 cores (B == 8). Per-core (S=2048, E=2048,
H=16, d=128):

Pass A (per head h, software-pipelined one head ahead):
  xb = one DMA per head carrying [xhT slab | xn slab] (packed on host)
  k = xhT-chunk-stationary @ Wk -> phi -> pk [s,d]
  q (quarter 0) = Wq-stationary @ xhT -> phi -> pq quarter tile
  Gram G[d',d] = sum_s x[s,d'] pk[s,d]; kv = (G*1024)^T @ Wv; ksum = pk^T@1
  kv_all[h] = [kv*1024 | ksum]   (ctx scale 1024 folded into kv via G copy)
  sc0's num/den start mid-pass-A once heads 0-7 are done.

Pass B (per s-chunk sc, pipelined one sc ahead; q quarters 1-3 drained
early so each quarter is complete before first use):
  nd[s, h-half, d] = pq-chunk^T @ kv_all (num) + den via ksum column
  inv = 1/den (DVE);  t = nd * inv_broadcast -> bf16 (= 1024*ctx)
  DMA-xbar transpose t -> tT [d, (h s)]
  c_hi = e4m3(tT) (ACT copy); c_lo = e4m3(tT - c_hi) (DVE sub)
  Wo in fp8 DoubleRow, 3 products per head-slab:
    instrA (per head): stationary (hi_h, lo_h), moving (Wo_hi_h bcast pair)
    instrB (head pair): stationary (hi_2j, hi_2j+1), moving (Wo_lo pair e5m2)
  acc -> bf16 out_sb (ACT) -> DRAM via Pool-dispatched DMA.
  Host: out/(1024*WS) + bias.
"""

import numpy as np
import ml_dtypes

import concourse.bass as bass
import concourse.mybir as mybir
import concourse.tile as tile
from concourse import bacc
from concourse.bass_utils import run_bass_kernel_spmd

S = 2048
E = 2048
H = 16
D = 128
N_CORES = 8
NCH = S // 128  # 16 s-chunks

F32 = mybir.dt.float32
BF16 = mybir.dt.bfloat16
E4 = mybir.dt.float8e4
E5 = mybir.dt.float8e5
AF = mybir.ActivationFunctionType
ALU = mybir.AluOpType
PM = mybir.MatmulPerfMode

CS = 1024.0   # ctx scale (folded into kv via Gram copy)
WS = 64.0     # Wo scale (host side)

_CACHED = {}


def build_module():
    nc = bacc.Bacc("TRN2", target_bir_lowering=False, debug=False,
                   num_devices=N_CORES)

    # xb[h] = [xhT slab h (128 x 2048) | xn slab h (128 x 2048)]
    xb = nc.dram_tensor("xb", [H, 128, 2 * S], BF16, kind="ExternalInput")
    wq = nc.dram_tensor("wq", [D, H * D], BF16, kind="ExternalInput")
    wkv = nc.dram_tensor("wkv", [D, H * 2 * D], BF16, kind="ExternalInput")
    woh = nc.dram_tensor("woh", [D, H, 4, 512], E4, kind="ExternalInput")
    wol = nc.dram_tensor("wol", [D, H, 4, 512], E5, kind="ExternalInput")
    out = nc.dram_tensor("out", [S, E], BF16, kind="ExternalOutput")

    with tile.TileContext(nc) as tc:
        with (
            tc.tile_pool(name="const", bufs=1) as const,
            tc.tile_pool(name="work", bufs=2) as work,
            tc.tile_pool(name="psum", bufs=2, space="PSUM") as psum,
        ):
            wq_sb = const.tile([128, H * D], BF16)
            nc.sync.dma_start(out=wq_sb[:], in_=wq[:])
            wkv_sb = const.tile([128, H * 2 * D], BF16)
            nc.sync.dma_start(out=wkv_sb[:, 0:256], in_=wkv[:, 0:256])
            nc.sync.dma_start(out=wkv_sb[:, 256:], in_=wkv[:, 256:])
            ones_col = const.tile([128, 1], BF16)
            nc.vector.memset(ones_col[:], 1.0)
            warm = const.tile([128, 1], F32)
            nc.vector.memset(warm[:], 0.0)
            nc.scalar.activation(warm[:], warm[:], AF.Exp)
            kv_all = const.tile([128, H * 129], BF16)  # [kv*CS | ksum]
            woh_sb = const.tile([128, H, 4, 512], E4)
            wol_sb = const.tile([128, H, 4, 512], E5)

            # rotating q-projection quarter tiles: pq[qtr][:, h*512 + s]
            q_tiles = {}

            def q_tile(qtr):
                if qtr not in q_tiles:
                    pq_quarter = work.tile([128, H * 512], BF16,
                                           tag="pqQ", bufs=2)
                    q_tiles[qtr] = pq_quarter
                return q_tiles[qtr]

            def phi_pre(psum_ap, n, relu_on_act=False):
                """exp + (1+relu) halves of phi; returns (e, t) tiles."""
                e = work.tile([128, 1024], BF16, tag="e", bufs=3)
                nc.scalar.activation(e[:, :n], psum_ap, AF.Exp)
                t = work.tile([128, 1024], BF16, tag="t", bufs=3)
                if relu_on_act:
                    r = work.tile([128, 1024], BF16, tag="r", bufs=2)
                    nc.scalar.activation(r[:, :n], psum_ap, AF.Relu)
                    nc.vector.tensor_scalar(t[:, :n], r[:, :n], 1.0, None,
                                            ALU.add)
                else:
                    nc.vector.tensor_scalar(t[:, :n], psum_ap, 0.0, 1.0,
                                            ALU.max, ALU.add)
                return e, t

            def phi_min(et, dst, n):
                e, t = et
                nc.vector.tensor_tensor(dst, e[:, :n], t[:, :n], ALU.min)

            def phi(psum_ap, dst, n, relu_on_act=False):
                """phi(x) = min(exp(x), 1+relu(x)); n = free size."""
                phi_min(phi_pre(psum_ap, n, relu_on_act), dst, n)

            def emit_q_chunk(h, qtr, xsrc=None, relu_on_act=False,
                             ptag="acc"):
                """q quarter matmul + phi -> quarter tile [:, h*512:+512]."""
                if xsrc is None:
                    xq = work.tile([128, 512], BF16, tag="xq", bufs=3)
                    nc.sync.dma_start(
                        out=xq[:],
                        in_=xb[h, :, qtr * 512:(qtr + 1) * 512])
                    src = xq[:]
                else:
                    src = xsrc
                qp = psum.tile([128, 512], F32, tag=ptag, bufs=2)
                nc.tensor.matmul(qp[:],
                                 wq_sb[:, h * 128:(h + 1) * 128],
                                 src, start=True, stop=True)
                pq = q_tile(qtr)
                phi(qp[:], pq[:, h * 512:(h + 1) * 512], 512, relu_on_act)

            # -------- Pass A (pipelined one head ahead) --------
            def emit_proj(h, mid_hook=None):
                xbh = work.tile([128, 2 * S], BF16, tag="xb", bufs=3)
                if h == 0:
                    nc.sync.dma_start(out=xbh[:, 0:512], in_=xb[0, :, 0:512])
                    nc.sync.dma_start(out=xbh[:, 512:], in_=xb[0, :, 512:])
                else:
                    nc.sync.dma_start(out=xbh[:], in_=xb[h])
                xn_v = xbh[:, S:].rearrange("p (c j) -> p c j", j=128)
                pk = work.tile([128, S], BF16, tag="pk", bufs=2)
                kps = []
                for j in range(2):
                    kp = psum.tile([128, 1024], F32, tag="pj", bufs=2)
                    for c in range(8):
                        sc = j * 8 + c
                        nc.tensor.matmul(
                            kp[:, c * 128:(c + 1) * 128],
                            xbh[:, sc * 128:(sc + 1) * 128],
                            wkv_sb[:, h * 256:h * 256 + 128],
                            start=True, stop=True)
                    kps.append(kp)
                    if j == 0 and mid_hook is not None:
                        mid_hook()
                qp = psum.tile([128, 512], F32, tag="acc", bufs=2)
                nc.tensor.matmul(qp[:], wq_sb[:, h * 128:(h + 1) * 128],
                                 xbh[:, 0:512], start=True, stop=True)
                # all PSUM-reads first (exp/ts), mins at the end: DVE never
                # blocks on ACT mid-chain
                ets = [phi_pre(kps[j][:], 1024, relu_on_act=(j == 1))
                       for j in range(2)]
                etq = phi_pre(qp[:], 512)
                for j in range(2):
                    phi_min(ets[j], pk[:, j * 1024:(j + 1) * 1024], 1024)
                pq = q_tile(0)
                phi_min(etq, pq[:, h * 512:(h + 1) * 512], 512)
                return pk, xn_v

            def emit_gram(h, pk, xn_v):
                gp = psum.tile([128, 512], F32, tag="g", bufs=2)
                kvp = psum.tile([128, 512], F32, tag="g", bufs=2)
                for c in range(NCH):
                    nc.tensor.matmul(gp[:, 0:128], xn_v[:, c, :],
                                     pk[:, c * 128:(c + 1) * 128],
                                     start=(c == 0), stop=(c == NCH - 1))
                    nc.tensor.matmul(kvp[:, 128:129],
                                     pk[:, c * 128:(c + 1) * 128],
                                     ones_col[:],
                                     start=(c == 0), stop=(c == NCH - 1))
                g_sb = work.tile([128, 128], BF16, tag="gsb", bufs=2)
                nc.scalar.activation(g_sb[:], gp[:, 0:128], AF.Copy, scale=CS)
                nc.tensor.matmul(kvp[:, 0:128], g_sb[:],
                                 wkv_sb[:, h * 256 + 128:h * 256 + 256],
                                 start=True, stop=True)
                nc.scalar.activation(
                    kv_all[:, h * 129:(h + 1) * 129], kvp[:, 0:129], AF.Copy)

            # -------- Pass B pieces --------
            tsc_tiles = {}
            cpair_tiles = {}

            def emit_norm1_half(sc, jh):
                """num/den -> inv -> t for heads jh*8..jh*8+7 of sc."""
                if sc not in tsc_tiles:
                    t_new = work.tile([128, H, 128], BF16,
                                      tag="tsc", bufs=3)
                    tsc_tiles[sc] = t_new
                t_sc = tsc_tiles[sc]
                pq = q_tiles[sc // 4]
                nd = psum.tile([128, 1024], F32, tag="pj", bufs=2)
                den = psum.tile([128, 512], F32, tag="g", bufs=2)
                for hh in range(8):
                    h = jh * 8 + hh
                    lhsT = pq[:, h * 512 + (sc % 4) * 128:
                              h * 512 + (sc % 4) * 128 + 128]
                    nc.tensor.matmul(
                        nd[:, hh * 128:(hh + 1) * 128], lhsT,
                        kv_all[:, h * 129:h * 129 + 128],
                        start=True, stop=True)
                    nc.tensor.matmul(
                        den[:, hh:hh + 1], lhsT,
                        kv_all[:, h * 129 + 128:(h + 1) * 129],
                        start=True, stop=True)
                inv = work.tile([128, 8], F32, tag="inv", bufs=4)
                nc.vector.reciprocal(inv[:], den[:, 0:8])
                nc.vector.tensor_tensor(
                    t_sc[:, jh * 8:(jh + 1) * 8, :],
                    nd[:].rearrange("p (hh j) -> p hh j", j=128),
                    inv[:].unsqueeze(2).to_broadcast((128, 8, 128)),
                    ALU.mult)

            def emit_norm2(sc):
                """transpose -> hi/lo split for sc."""
                t_sc = tsc_tiles.pop(sc)
                tT = work.tile([128, H, 128], BF16, tag="tsc", bufs=3)
                nc.sync.dma_start(out=tT[:],
                                  in_=t_sc[:].rearrange("p h j -> p (h j)"),
                                  transpose=True)
                cpair = work.tile([128, 2, H, 128], E4, tag="cp", bufs=3)
                nc.scalar.activation(cpair[:, 0], tT[:], AF.Copy)
                nc.vector.tensor_tensor(cpair[:, 1], tT[:], cpair[:, 0],
                                        ALU.subtract)
                cpair_tiles[sc] = cpair

            def emit_wo(sc, eop):
                cpair = cpair_tiles[sc]
                acc = psum.tile([128, 512], F32, tag="acc", bufs=2)
                for h in range(H):
                    nc.tensor.matmul(
                        acc[:], cpair[:, :, h, :],
                        woh_sb[:, h, eop].unsqueeze(1).to_broadcast(
                            (128, 2, 512)),
                        start=(h == 0), stop=False,
                        perf_mode=PM.DoubleRow)
                for j in range(8):
                    nc.tensor.matmul(
                        acc[:], cpair[:, 0, 2 * j:2 * j + 2, :],
                        wol_sb[:, 2 * j:2 * j + 2, eop],
                        start=False, stop=(j == 7),
                        perf_mode=PM.DoubleRow)
                out_sb = work.tile([128, 512], BF16, tag="osb", bufs=3)
                nc.scalar.activation(out_sb[:], acc[:], AF.Copy)
                # Pool-dispatched (SWDGE): its wait doesn't block SP queue
                nc.gpsimd.dma_start(
                    out=out[sc * 128:(sc + 1) * 128,
                            eop * 512:(eop + 1) * 512],
                    in_=out_sb[:])

            q_queue = [(h, qtr) for qtr in (1, 2, 3) for h in range(H)]
            q_queue.reverse()  # pop from the end = in-order

            def drain_q(n):
                for _ in range(n):
                    if q_queue:
                        h, qtr = q_queue.pop()
                        emit_q_chunk(h, qtr, relu_on_act=(h % 2 == 0),
                                     ptag="g")

            # ---- Pass A main loop ----
            prev = None
            for h in range(H):
                pv = prev
                cur = emit_proj(
                    h, mid_hook=(
                        (lambda: emit_gram(h - 1, *pv)) if pv else None))
                prev = cur
                if h == 9:
                    # heads 0-7 kv ready: start sc0's first half
                    emit_norm1_half(0, 0)
                if h == 11:
                    emit_norm1_half(1, 0)
                if h == 12:
                    nc.sync.dma_start(out=woh_sb[:], in_=woh[:])
                if h == 14:
                    nc.sync.dma_start(out=wol_sb[:], in_=wol[:])
            emit_gram(H - 1, *prev)

            # ---- Pass B ----
            emit_norm1_half(0, 1)
            drain_q(3)
            emit_norm2(0)
            emit_norm1_half(1, 1)
            drain_q(3)
            for sc in range(1, NCH + 1):
                if sc < NCH:
                    emit_norm2(sc)
                drain_q(2)
                if sc + 1 < NCH:
                    emit_norm1_half(sc + 1, 0)
                emit_wo(sc - 1, 0)
                drain_q(2)
                emit_wo(sc - 1, 1)
                emit_wo(sc - 1, 2)
                if sc + 1 < NCH:
                    emit_norm1_half(sc + 1, 1)
                emit_wo(sc - 1, 3)
                del cpair_tiles[sc - 1]
                if sc % 4 == 3 and (sc // 4) in q_tiles:
                    del q_tiles[sc // 4]
            drain_q(48)  # flush (no-op when queue empty)

    nc.compile()
    return nc


def get_module():
    if "nc" not in _CACHED:
        _CACHED["nc"] = build_module()
    return _CACHED["nc"]


def _bf16(a):
    return np.ascontiguousarray(a).astype(ml_dtypes.bfloat16)


def prepare_in_maps(inputs, Wq, Wk, Wv, Wo, bo):
    """Host-side shard + layout prep. Returns per-core input maps."""
    wq_p = _bf16(np.transpose(np.asarray(Wq), (1, 0, 2)).reshape(D, H * D))
    wkv = np.concatenate([np.asarray(Wk), np.asarray(Wv)], axis=2)  # (H,d,2d)
    wkv_p = _bf16(np.transpose(wkv, (1, 0, 2)).reshape(D, H * 2 * D))
    wo_t = np.transpose(np.asarray(Wo).reshape(H, D, E), (1, 0, 2)) * WS
    woh_p = np.ascontiguousarray(
        wo_t.reshape(D, H, 4, 512)).astype(ml_dtypes.float8_e4m3)
    wol_p = np.ascontiguousarray(
        (wo_t - woh_p.astype(np.float32).reshape(D, H, E)
         ).reshape(D, H, 4, 512)).astype(ml_dtypes.float8_e5m2)
    in_maps = []
    for b in range(N_CORES):
        xbm = np.asarray(inputs[b])
        # xhT slab h: x[:, h*128:(h+1)*128].T ; xn slab h: natural layout
        xt = np.transpose(xbm.reshape(S, H, D), (1, 2, 0))        # H, D, S
        xnat = np.transpose(xbm.reshape(NCH, 128, H, D),
                            (2, 1, 0, 3)).reshape(H, 128, S)      # H,128,S
        xb_p = _bf16(np.concatenate([xt, xnat], axis=2))          # H,128,2S
        in_maps.append({"xb": xb_p,
                        "wq": wq_p, "wkv": wkv_p,
                        "woh": woh_p, "wol": wol_p})
    return in_maps


def kernel(inputs, Wq, Wk, Wv, Wo, bo):
    B = inputs.shape[0]
    assert B == N_CORES and inputs.shape[1:] == (S, E)
    nc = get_module()
    in_maps = prepare_in_maps(inputs, Wq, Wk, Wv, Wo, bo)
    res = run_bass_kernel_spmd(nc, in_maps, list(range(N_CORES)))
    outs = np.stack([res.results[b]["out"].astype(np.float32)
                     for b in range(N_CORES)], axis=0)
    outs /= (CS * WS)
    return (outs + np.asarray(bo, dtype=np.float32)[None, None, :]).astype(
        np.float32)
